# revision 12
# baseline (speedup 1.0000x reference)
"""AdaPT int8-quantized Linear on 8 TRN2 NeuronCores.

out = round_int8(x * 127/amax(x)) @ round_int8(w * 127/amax(w)).T * dequant + bias

Exactness: int8 values (|v| <= 127) are exact in bf16; their products
(<= 16129) and the accumulated partial sums are exact in fp32 PSUM, so a
bf16 TensorE matmul reproduces the int8 x int8 -> int32 matmul bit-exactly
at full bf16 throughput. round() is implemented as (v*scale + 1.5*2^23) -
1.5*2^23 in f32 (round-half-even, matching jnp) on VectorE only (ScalarE's
activation affine pre-op is not exact; GpSimd tensor_scalar contends with
DVE for the shared SBUF port pair and fully blocks -- measured 25x).

Pipeline (v5):
- Loads ride three DMA queues, amax bytes first on each:
  sync  [x-amax 0-7 | x re-read x16]
  scalar[x-amax 8-15| bias | w panel 0..7]
  gpsimd[w-amax 0-7]           (SWDGE; safe: V runs only tensor_reduce then)
- A primer AllReduce issued at t~0 absorbs the CC engine's ~45-65us
  first-collective cold start while the amax DMA streams; ONE combined
  AllReduce-max then carries [amax_x, amax_w] on the warm path (~10us).
- Scales broadcast via a partition-replicating DMA read of the AR result,
  queued on gpsimd between keep-alive ops so it fires as the AR lands.
- x re-read lands n-block-major into a block-contiguous xT
  [P][nb][ks][128] so every Vector op is one contiguous run (strided
  sub-row writes measured 16x slower on DVE).
- Vector's queue is strict FIFO, so all early-ready quant work for the
  NEXT panel is emitted in a pre-block before this panel's PSUM-gated
  epilogues; epilogues then interleave at matmul pace without blocking.
- Paced dummy matmuls (on amax partials, then a GpSimd tensor_tensor
  keep-alive chain -- tensor_tensor never takes the shared port) hold the
  PE HAM warm through the prologue.

x row-parallel: core c computes out rows [c*1024, (c+1)*1024).
"""

import numpy as np

import concourse.bass as bass
import concourse.bacc as bacc
import concourse.bass_isa as bass_isa
import concourse.mybir as mybir
import concourse.tile as tile
from concourse.bass_utils import run_bass_kernel_spmd

N, K, M = 8192, 4096, 4096
N_CORES = 8
NS = N // N_CORES   # 1024 x rows per core
WS = M // N_CORES   # 512 w rows per core (amax shard)
P = 128
KB = K // P         # 32 k-blocks
NB = NS // P        # 8 n-blocks per core
MP = 512            # m-panel width
NMP = M // MP       # 8 m-panels

MAGIC = 12582912.0  # 1.5 * 2**23
F32 = mybir.dt.float32
BF16 = mybir.dt.bfloat16

_cached_nc = None


def _body(nc, tc, xs, wa, wf, bias_in, out):
    RG = [list(range(N_CORES))]
    xa_t = xs.rearrange("(t a p) n -> t p a n", a=2, p=P)   # [16, 128, 2, 1024]
    wa_t = wa.rearrange("(h a p) m -> h p a m", a=4, p=P)   # [8, 128, 4, 512]

    with (
        tc.tile_pool(name="const", bufs=1) as const,
        tc.tile_pool(name="dram", bufs=1, space="DRAM") as dram,
        tc.tile_pool(name="ld", bufs=6) as ld,
        tc.tile_pool(name="xt", bufs=1) as xtp,
        tc.tile_pool(name="wt", bufs=8) as wtp,
        tc.tile_pool(name="ps", bufs=7, space="PSUM") as psp,
        tc.tile_pool(name="psd", bufs=1, space="PSUM") as psdp,
        tc.tile_pool(name="ob", bufs=4) as obp,
    ):
        ccp_in = dram.tile([1, 16], F32)
        ccp_out = dram.tile([1, 16], F32, addr_space="Shared")
        ccm_in = dram.tile([1, 16], F32)
        ccm_out = dram.tile([1, 16], F32, addr_space="Shared")
        bias_bc = const.tile([P, M], F32)
        scl = const.tile([P, 4], F32)   # 0:scale_x 1:scale_w 2:dequant 3:tmp
        psd = psdp.tile([P, 16], F32)   # dummy-matmul scratch bank
        junk = const.tile([P, 1024], F32)

        def dummy_mm(dep_ap):
            # Tiny matmul reading dep_ap: paced by dep_ap's producer, keeps
            # the PE HAM activity window warm before the main loop.
            nc.tensor.matmul(psd[0:1, 0:1], dep_ap, dep_ap,
                             start=True, stop=True)

        # ---- primer collective: absorbs CC first-mesh cold start ----
        primer = const.tile([1, 16], F32)
        nc.vector.memset(primer[:], 0.0)
        nc.vector.memset(junk[:], 1.0)
        nc.gpsimd.dma_start(ccp_in[:], primer[:])
        nc.gpsimd.collective_compute(
            "AllReduce", mybir.AluOpType.max,
            ins=[ccp_in.opt()], outs=[ccp_out.opt()], replica_groups=RG,
        )

        # ---- abs-max loads: three queues, amax bytes first ----
        partw = const.tile([P, 8], F32)
        partx = const.tile([P, 16], F32)
        wa_tiles = []
        for h in range(8):
            tl = ld.tile([P, 4, WS], F32, tag="ld", name=f"ldwa{h}")
            nc.gpsimd.dma_start(tl[:], wa_t[h])
            wa_tiles.append(tl)
        xa_tiles = []
        for t in range(8):
            tl = ld.tile([P, 2, NS], F32, tag="ld", name=f"ldxa{t}")
            nc.sync.dma_start(tl[:], xa_t[t])
            xa_tiles.append(tl)
        for t in range(8, 16):
            tl = ld.tile([P, 2, NS], F32, tag="ld", name=f"ldxb{t}")
            nc.scalar.dma_start(tl[:], xa_t[t])
            xa_tiles.append(tl)

        # reduces interleaved in expected arrival order
        for i in range(8):
            nc.vector.tensor_reduce(
                out=partw[:, i : i + 1], in_=wa_tiles[i][:],
                op=mybir.AluOpType.max, axis=mybir.AxisListType.XY,
                apply_absolute_value=True,
            )
            dummy_mm(partw[:, i : i + 1])
            for j in (i, 8 + i):
                nc.vector.tensor_reduce(
                    out=partx[:, j : j + 1], in_=xa_tiles[j][:],
                    op=mybir.AluOpType.max, axis=mybir.AxisListType.XY,
                    apply_absolute_value=True,
                )
                dummy_mm(partx[:, j : j + 1])

        pw = const.tile([P, 1], F32)
        nc.vector.tensor_reduce(out=pw[:], in_=partw[:], op=mybir.AluOpType.max,
                                axis=mybir.AxisListType.X)
        px = const.tile([P, 1], F32)
        nc.vector.tensor_reduce(out=px[:], in_=partx[:], op=mybir.AluOpType.max,
                                axis=mybir.AxisListType.X)
        rw = const.tile([P, 1], F32)
        nc.gpsimd.partition_all_reduce(rw[:], pw[:], channels=P,
                                       reduce_op=bass_isa.ReduceOp.max)
        rx = const.tile([P, 1], F32)
        nc.gpsimd.partition_all_reduce(rx[:], px[:], channels=P,
                                       reduce_op=bass_isa.ReduceOp.max)
        packm = const.tile([1, 16], F32)
        nc.vector.memset(packm[:], 0.0)
        nc.vector.tensor_copy(packm[:1, 0:1], rx[:1, :])
        nc.vector.tensor_copy(packm[:1, 1:2], rw[:1, :])
        nc.gpsimd.dma_start(ccm_in[:], packm[:])
        nc.gpsimd.collective_compute(
            "AllReduce", mybir.AluOpType.max,
            ins=[ccm_in.opt()], outs=[ccm_out.opt()], replica_groups=RG,
        )

        # keep-alive chain on GpSimd (tensor_tensor never contends with
        # DVE) pacing dummy matmuls across the collective's flight; the
        # AR-result readback rides the same queue mid-chain so it fires
        # the moment the collective lands
        def keep_alive(i):
            half = 512
            src = junk[:, (i % 2) * half : (i % 2) * half + half]
            dst = junk[:, (1 - i % 2) * half : (1 - i % 2) * half + half]
            nc.gpsimd.tensor_tensor(out=dst, in0=src, in1=src,
                                    op=mybir.AluOpType.add)
            dummy_mm(dst[:, 0:1])

        for i in range(20):
            keep_alive(i)

        gbm = const.tile([P, 16], F32)
        ccm_ap0 = ccm_out[:]
        ccm_b_ap = bass.AP(
            tensor=ccm_ap0.tensor,
            offset=ccm_ap0.offset,
            ap=[[0, P]] + [list(d) for d in ccm_ap0.ap][-1:],
        )
        nc.gpsimd.dma_start(out=gbm[:], in_=ccm_b_ap)
        for i in range(4):
            keep_alive(20 + i)

        # ---- scales ----
        invx = const.tile([P, 1], F32)
        nc.vector.reciprocal(invx[:], gbm[:, 0:1])
        nc.vector.tensor_scalar(out=scl[:, 0:1], in0=invx[:], scalar1=127.0,
                                scalar2=None, op0=mybir.AluOpType.mult)
        invw = const.tile([P, 1], F32)
        nc.vector.reciprocal(invw[:], gbm[:, 1:2])
        nc.vector.tensor_scalar(out=scl[:, 1:2], in0=invw[:], scalar1=127.0,
                                scalar2=None, op0=mybir.AluOpType.mult)
        nc.vector.tensor_tensor(out=scl[:, 3:4], in0=gbm[:, 0:1], in1=gbm[:, 1:2],
                                op=mybir.AluOpType.mult)
        nc.vector.tensor_scalar(out=scl[:, 2:3], in0=scl[:, 3:4],
                                scalar1=float(np.float32(1.0) / np.float32(16129.0)),
                                scalar2=None, op0=mybir.AluOpType.mult)
        dummy_mm(scl[:, 2:3])

        # bias broadcast load (scalar queue, needed by first epilogue)
        bias_b_ap = bass.AP(
            tensor=bias_in.tensor,
            offset=bias_in.offset,
            ap=[[0, P]] + list(bias_in.ap),
        )
        nc.scalar.dma_start(out=bias_bc[:], in_=bias_b_ap)

        # ---- w panels: load (scalar queue) + quantize (vector) ----
        def load_panel(p):
            tls = []
            for g in range(8):
                tl = ld.tile([P, 4, MP], F32, tag="ld", name=f"wld{p}_{g}")
                src = bass.AP(
                    tensor=wf.tensor,
                    offset=wf.offset + g * (K // 8) * M + p * MP,
                    ap=[[M, P], [P * M, 4], [1, MP]],
                )
                nc.scalar.dma_start(tl[:], src)
                tls.append(tl)
            return tls

        def quant_chunk(p, g, tl, wq, warm=False):
            nc.vector.tensor_scalar(out=tl[:], in0=tl[:], scalar1=scl[:, 1:2],
                                    scalar2=MAGIC, op0=mybir.AluOpType.mult,
                                    op1=mybir.AluOpType.add)
            dst = wq[g // 2][:, 4 * (g % 2) : 4 * (g % 2) + 4, :]
            nc.vector.tensor_scalar(out=dst, in0=tl[:], scalar1=MAGIC,
                                    scalar2=None, op0=mybir.AluOpType.subtract)
            if warm:
                dummy_mm(wq[g // 2][:, 4 * (g % 2), 0:1])

        # ---- x re-read (sync queue), quantize into blocked xT ----
        # xT layout [P][nb][ks][128]: per-nb writes are one contiguous run.
        # Creation order matches Vector consumption order so the 6-buffer
        # ld pool's WAR recycling never blocks an urgent DMA.
        xT = xtp.tile([P, NB, KB, P], BF16)  # resident quantized x.T (8.4 MB)
        xre_tiles = [None] * NB

        def load_xre(nb):
            halves = []
            for a in range(2):
                tl = ld.tile([P, 16, P], F32, tag="ld", name=f"xre{nb}_{a}")
                src = bass.AP(
                    tensor=xs.tensor,
                    offset=xs.offset + a * 16 * P * NS + nb * P,
                    ap=[[NS, P], [P * NS, 16], [1, P]],
                )
                nc.sync.dma_start(tl[:], src)
                halves.append(tl)
            xre_tiles[nb] = halves

        load_xre(0)
        load_xre(1)
        wq_cur = [wtp.tile([P, 8, MP], BF16, tag="wq", name=f"wq0_{h}")
                  for h in range(4)]
        tls0 = load_panel(0)
        for nb in range(2, NB):
            load_xre(nb)

        def quant_x(nb):
            for a in range(2):
                tl = xre_tiles[nb][a]
                nc.vector.tensor_scalar(out=tl[:], in0=tl[:], scalar1=scl[:, 0:1],
                                        scalar2=MAGIC, op0=mybir.AluOpType.mult,
                                        op1=mybir.AluOpType.add)
                nc.vector.tensor_scalar(
                    out=xT[:, nb, a * 16 : a * 16 + 16, :],
                    in0=tl[:], scalar1=MAGIC, scalar2=None,
                    op0=mybir.AluOpType.subtract)

        # panel-0 pre-block: everything Vector must finish for panel 0 and
        # panel 1, in readiness order, ahead of any PSUM-gated epilogue
        quant_x(0)
        quant_chunk(0, 0, tls0[0], wq_cur, warm=True)
        quant_x(1)
        for g in range(1, 8):
            quant_chunk(0, g, tls0[g], wq_cur, warm=True)
        for nb in range(2, NB):
            quant_x(nb)
        tls1 = load_panel(1)
        wq1 = [wtp.tile([P, 8, MP], BF16, tag="wq", name=f"wq1_{h}")
               for h in range(4)]
        for g in range(8):
            quant_chunk(1, g, tls1[g], wq1)

        # ---- main loop: 8 panels x 8 n-blocks x 32 k-steps ----
        wq_next, tls_next = wq1, tls1
        for p in range(NMP):
            if p >= 1 and p + 1 < NMP:
                # pre-block: next panel's quantize, ahead of epilogues
                tls_next = load_panel(p + 1)
                wq_next = [wtp.tile([P, 8, MP], BF16, tag="wq",
                                    name=f"wq{p + 1}_{h}") for h in range(4)]
                for g in range(8):
                    quant_chunk(p + 1, g, tls_next[g], wq_next)
            for nb in range(NB):
                ps = psp.tile([P, MP], F32, tag="ps", name=f"ps{p}_{nb}")
                for i in range(KB):
                    ks = (4 * nb + i) % KB
                    nc.tensor.matmul(
                        ps[:], xT[:, nb, ks, :],
                        wq_cur[ks // 8][:, ks % 8, :],
                        start=(i == 0), stop=(i == KB - 1),
                    )
                ob = obp.tile([P, MP], F32, tag="ob", name=f"ob{p}_{nb}")
                nc.vector.scalar_tensor_tensor(
                    out=ob[:], in0=ps[:], scalar=scl[:, 2:3],
                    in1=bias_bc[:, p * MP : (p + 1) * MP],
                    op0=mybir.AluOpType.mult, op1=mybir.AluOpType.add,
                )
                nc.gpsimd.dma_start(
                    out[nb * P : (nb + 1) * P, p * MP : (p + 1) * MP], ob[:]
                )
            wq_cur = wq_next


def _build():
    global _cached_nc
    if _cached_nc is not None:
        return _cached_nc
    nc = bacc.Bacc("TRN2", target_bir_lowering=False, debug=False,
                   num_devices=N_CORES)
    xs = nc.dram_tensor("xs", [K, NS], F32, kind="ExternalInput")
    wa = nc.dram_tensor("wa", [K, WS], F32, kind="ExternalInput")
    wf = nc.dram_tensor("wf", [K, M], F32, kind="ExternalInput")
    bias = nc.dram_tensor("bias", [M], F32, kind="ExternalInput")
    out = nc.dram_tensor("out", [NS, M], F32, kind="ExternalOutput")
    with tile.TileContext(nc) as tc:
        _body(nc, tc, xs.ap(), wa.ap(), wf.ap(), bias.ap(), out.ap())
    nc.compile()
    _cached_nc = nc
    return nc


def kernel(x, weight, bias, _trace=False, _trace_kwargs=None):
    x = np.asarray(x, dtype=np.float32)
    weight = np.asarray(weight, dtype=np.float32)
    bias = np.ascontiguousarray(np.asarray(bias, dtype=np.float32))
    assert x.shape == (N, K) and weight.shape == (M, K) and bias.shape == (M,)

    nc = _build()
    xt = x.T                              # [K, N] view
    wt = np.ascontiguousarray(weight.T)   # [K, M]
    in_maps = [
        {
            "xs": np.ascontiguousarray(xt[:, c * NS : (c + 1) * NS]),
            "wa": np.ascontiguousarray(wt[:, c * WS : (c + 1) * WS]),
            "wf": wt,
            "bias": bias,
        }
        for c in range(N_CORES)
    ]
    res = run_bass_kernel_spmd(
        nc, in_maps, core_ids=list(range(N_CORES)),
        trace=_trace, **(_trace_kwargs or {}),
    )
    out = np.concatenate([res.results[c]["out"] for c in range(N_CORES)], axis=0)
    if _trace:
        return out, res
    return out


# revision 13
# speedup vs baseline: 1.0173x; 1.0173x over previous
"""AdaPT int8-quantized Linear on 8 TRN2 NeuronCores.

out = round_int8(x * 127/amax(x)) @ round_int8(w * 127/amax(w)).T * dequant + bias

Exactness: int8 values (|v| <= 127) are exact in bf16; their products
(<= 16129) and the accumulated partial sums are exact in fp32 PSUM, so a
bf16 TensorE matmul reproduces the int8 x int8 -> int32 matmul bit-exactly
at full bf16 throughput. round() is implemented as (v*scale + 1.5*2^23) -
1.5*2^23 in f32 (round-half-even, matching jnp) on VectorE only (ScalarE's
activation affine pre-op is not exact; GpSimd tensor_scalar contends with
DVE for the shared SBUF port pair and fully blocks -- measured 25x).

Pipeline (v5):
- Loads ride three DMA queues, amax bytes first on each:
  sync  [x-amax 0-7 | x re-read x16]
  scalar[x-amax 8-15| bias | w panel 0..7]
  gpsimd[w-amax 0-7]           (SWDGE; safe: V runs only tensor_reduce then)
- A primer AllReduce issued at t~0 absorbs the CC engine's ~45-65us
  first-collective cold start while the amax DMA streams; ONE combined
  AllReduce-max then carries [amax_x, amax_w] on the warm path (~10us).
- Scales broadcast via a partition-replicating DMA read of the AR result,
  queued on gpsimd between keep-alive ops so it fires as the AR lands.
- x re-read lands n-block-major into a block-contiguous xT
  [P][nb][ks][128] so every Vector op is one contiguous run (strided
  sub-row writes measured 16x slower on DVE).
- Vector's queue is strict FIFO, so all early-ready quant work for the
  NEXT panel is emitted in a pre-block before this panel's PSUM-gated
  epilogues; epilogues then interleave at matmul pace without blocking.
- Paced dummy matmuls (on amax partials, then a GpSimd tensor_tensor
  keep-alive chain -- tensor_tensor never takes the shared port) hold the
  PE HAM warm through the prologue.

x row-parallel: core c computes out rows [c*1024, (c+1)*1024).
"""

import numpy as np

import concourse.bass as bass
import concourse.bacc as bacc
import concourse.bass_isa as bass_isa
import concourse.mybir as mybir
import concourse.tile as tile
from concourse.bass_utils import run_bass_kernel_spmd

N, K, M = 8192, 4096, 4096
N_CORES = 8
NS = N // N_CORES   # 1024 x rows per core
WS = M // N_CORES   # 512 w rows per core (amax shard)
P = 128
KB = K // P         # 32 k-blocks
NB = NS // P        # 8 n-blocks per core
MP = 512            # m-panel width
NMP = M // MP       # 8 m-panels

MAGIC = 12582912.0  # 1.5 * 2**23
F32 = mybir.dt.float32
BF16 = mybir.dt.bfloat16

_cached_nc = None


def _body(nc, tc, xs, wa, wf, bias_in, out):
    RG = [list(range(N_CORES))]
    xa_t = xs.rearrange("(t a p) n -> t p a n", a=2, p=P)   # [16, 128, 2, 1024]
    wa_t = wa.rearrange("(h a p) m -> h p a m", a=4, p=P)   # [8, 128, 4, 512]

    with (
        tc.tile_pool(name="const", bufs=1) as const,
        tc.tile_pool(name="dram", bufs=1, space="DRAM") as dram,
        tc.tile_pool(name="ld", bufs=6) as ld,
        tc.tile_pool(name="xt", bufs=1) as xtp,
        tc.tile_pool(name="wt", bufs=8) as wtp,
        tc.tile_pool(name="ps", bufs=7, space="PSUM") as psp,
        tc.tile_pool(name="psd", bufs=1, space="PSUM") as psdp,
        tc.tile_pool(name="ob", bufs=4) as obp,
    ):
        ccp_in = dram.tile([1, 16], F32)
        ccp_out = dram.tile([1, 16], F32, addr_space="Shared")
        ccm_in = dram.tile([1, 16], F32)
        ccm_out = dram.tile([1, 16], F32, addr_space="Shared")
        bias_bc = const.tile([P, M], F32)
        scl = const.tile([P, 4], F32)   # 0:scale_x 1:scale_w 2:dequant 3:tmp
        psd = psdp.tile([P, 16], F32)   # dummy-matmul scratch bank
        junk = const.tile([P, 1024], F32)

        def dummy_mm(dep_ap):
            # Tiny matmul reading dep_ap: paced by dep_ap's producer, keeps
            # the PE HAM activity window warm before the main loop.
            nc.tensor.matmul(psd[0:1, 0:1], dep_ap, dep_ap,
                             start=True, stop=True)

        # ---- primer collective: absorbs CC first-mesh cold start ----
        primer = const.tile([1, 16], F32)
        nc.vector.memset(primer[:], 0.0)
        nc.vector.memset(junk[:], 1.0)
        nc.gpsimd.dma_start(ccp_in[:], primer[:])
        nc.gpsimd.collective_compute(
            "AllReduce", mybir.AluOpType.max,
            ins=[ccp_in.opt()], outs=[ccp_out.opt()], replica_groups=RG,
        )

        # ---- abs-max loads: three queues, amax bytes first ----
        partw = const.tile([P, 8], F32)
        partx = const.tile([P, 16], F32)
        xa_tiles = []
        for t in range(8):
            tl = ld.tile([P, 2, NS], F32, tag="ld", name=f"ldxa{t}")
            nc.sync.dma_start(tl[:], xa_t[t])
            xa_tiles.append(tl)
        for t in range(8, 16):
            tl = ld.tile([P, 2, NS], F32, tag="ld", name=f"ldxb{t}")
            nc.scalar.dma_start(tl[:], xa_t[t])
            xa_tiles.append(tl)
        wa_tiles = []
        for h in range(4):
            tl = ld.tile([P, 4, WS], F32, tag="ld", name=f"ldwa{h}")
            nc.sync.dma_start(tl[:], wa_t[h])
            wa_tiles.append(tl)
        for h in range(4, 8):
            tl = ld.tile([P, 4, WS], F32, tag="ld", name=f"ldwb{h}")
            nc.scalar.dma_start(tl[:], wa_t[h])
            wa_tiles.append(tl)

        # reduces interleaved in expected arrival order
        for i in range(8):
            for j in (i, 8 + i):
                nc.vector.tensor_reduce(
                    out=partx[:, j : j + 1], in_=xa_tiles[j][:],
                    op=mybir.AluOpType.max, axis=mybir.AxisListType.XY,
                    apply_absolute_value=True,
                )
                dummy_mm(partx[:, j : j + 1])
            nc.vector.tensor_reduce(
                out=partw[:, i : i + 1], in_=wa_tiles[i][:],
                op=mybir.AluOpType.max, axis=mybir.AxisListType.XY,
                apply_absolute_value=True,
            )
            dummy_mm(partw[:, i : i + 1])

        pw = const.tile([P, 1], F32)
        nc.vector.tensor_reduce(out=pw[:], in_=partw[:], op=mybir.AluOpType.max,
                                axis=mybir.AxisListType.X)
        px = const.tile([P, 1], F32)
        nc.vector.tensor_reduce(out=px[:], in_=partx[:], op=mybir.AluOpType.max,
                                axis=mybir.AxisListType.X)
        rw = const.tile([P, 1], F32)
        nc.gpsimd.partition_all_reduce(rw[:], pw[:], channels=P,
                                       reduce_op=bass_isa.ReduceOp.max)
        rx = const.tile([P, 1], F32)
        nc.gpsimd.partition_all_reduce(rx[:], px[:], channels=P,
                                       reduce_op=bass_isa.ReduceOp.max)
        packm = const.tile([1, 16], F32)
        nc.vector.memset(packm[:], 0.0)
        nc.vector.tensor_copy(packm[:1, 0:1], rx[:1, :])
        nc.vector.tensor_copy(packm[:1, 1:2], rw[:1, :])
        nc.gpsimd.dma_start(ccm_in[:], packm[:])
        nc.gpsimd.collective_compute(
            "AllReduce", mybir.AluOpType.max,
            ins=[ccm_in.opt()], outs=[ccm_out.opt()], replica_groups=RG,
        )

        # keep-alive chain on GpSimd (tensor_tensor never contends with
        # DVE) pacing dummy matmuls across the collective's flight; the
        # AR-result readback rides the same queue mid-chain so it fires
        # the moment the collective lands
        def keep_alive(i):
            half = 512
            src = junk[:, (i % 2) * half : (i % 2) * half + half]
            dst = junk[:, (1 - i % 2) * half : (1 - i % 2) * half + half]
            nc.gpsimd.tensor_tensor(out=dst, in0=src, in1=src,
                                    op=mybir.AluOpType.add)
            dummy_mm(dst[:, 0:1])

        for i in range(20):
            keep_alive(i)

        gbm = const.tile([P, 16], F32)
        ccm_ap0 = ccm_out[:]
        ccm_b_ap = bass.AP(
            tensor=ccm_ap0.tensor,
            offset=ccm_ap0.offset,
            ap=[[0, P]] + [list(d) for d in ccm_ap0.ap][-1:],
        )
        nc.gpsimd.dma_start(out=gbm[:], in_=ccm_b_ap)
        for i in range(4):
            keep_alive(20 + i)

        # ---- scales ----
        invx = const.tile([P, 1], F32)
        nc.vector.reciprocal(invx[:], gbm[:, 0:1])
        nc.vector.tensor_scalar(out=scl[:, 0:1], in0=invx[:], scalar1=127.0,
                                scalar2=None, op0=mybir.AluOpType.mult)
        invw = const.tile([P, 1], F32)
        nc.vector.reciprocal(invw[:], gbm[:, 1:2])
        nc.vector.tensor_scalar(out=scl[:, 1:2], in0=invw[:], scalar1=127.0,
                                scalar2=None, op0=mybir.AluOpType.mult)
        nc.vector.tensor_tensor(out=scl[:, 3:4], in0=gbm[:, 0:1], in1=gbm[:, 1:2],
                                op=mybir.AluOpType.mult)
        nc.vector.tensor_scalar(out=scl[:, 2:3], in0=scl[:, 3:4],
                                scalar1=float(np.float32(1.0) / np.float32(16129.0)),
                                scalar2=None, op0=mybir.AluOpType.mult)
        dummy_mm(scl[:, 2:3])

        # bias broadcast load (scalar queue, needed by first epilogue)
        bias_b_ap = bass.AP(
            tensor=bias_in.tensor,
            offset=bias_in.offset,
            ap=[[0, P]] + list(bias_in.ap),
        )
        nc.scalar.dma_start(out=bias_bc[:], in_=bias_b_ap)

        # ---- w panels: load (scalar queue) + quantize (vector) ----
        def load_panel(p):
            tls = []
            for g in range(8):
                tl = ld.tile([P, 4, MP], F32, tag="ld", name=f"wld{p}_{g}")
                src = bass.AP(
                    tensor=wf.tensor,
                    offset=wf.offset + g * (K // 8) * M + p * MP,
                    ap=[[M, P], [P * M, 4], [1, MP]],
                )
                nc.scalar.dma_start(tl[:], src)
                tls.append(tl)
            return tls

        def quant_chunk(p, g, tl, wq, warm=False):
            nc.vector.tensor_scalar(out=tl[:], in0=tl[:], scalar1=scl[:, 1:2],
                                    scalar2=MAGIC, op0=mybir.AluOpType.mult,
                                    op1=mybir.AluOpType.add)
            dst = wq[g // 2][:, 4 * (g % 2) : 4 * (g % 2) + 4, :]
            nc.vector.tensor_scalar(out=dst, in0=tl[:], scalar1=MAGIC,
                                    scalar2=None, op0=mybir.AluOpType.subtract)
            if warm:
                dummy_mm(wq[g // 2][:, 4 * (g % 2), 0:1])

        # ---- x re-read (sync queue), quantize into blocked xT ----
        # xT layout [P][nb][ks][128]: per-nb writes are one contiguous run.
        # Creation order matches Vector consumption order so the 6-buffer
        # ld pool's WAR recycling never blocks an urgent DMA.
        xT = xtp.tile([P, NB, KB, P], BF16)  # resident quantized x.T (8.4 MB)
        xre_tiles = [None] * NB

        def load_xre(nb):
            halves = []
            for a in range(2):
                tl = ld.tile([P, 16, P], F32, tag="ld", name=f"xre{nb}_{a}")
                src = bass.AP(
                    tensor=xs.tensor,
                    offset=xs.offset + a * 16 * P * NS + nb * P,
                    ap=[[NS, P], [P * NS, 16], [1, P]],
                )
                nc.sync.dma_start(tl[:], src)
                halves.append(tl)
            xre_tiles[nb] = halves

        load_xre(0)
        load_xre(1)
        wq_cur = [wtp.tile([P, 8, MP], BF16, tag="wq", name=f"wq0_{h}")
                  for h in range(4)]
        tls0 = load_panel(0)
        for nb in range(2, NB):
            load_xre(nb)

        def quant_x(nb):
            for a in range(2):
                tl = xre_tiles[nb][a]
                nc.vector.tensor_scalar(out=tl[:], in0=tl[:], scalar1=scl[:, 0:1],
                                        scalar2=MAGIC, op0=mybir.AluOpType.mult,
                                        op1=mybir.AluOpType.add)
                nc.vector.tensor_scalar(
                    out=xT[:, nb, a * 16 : a * 16 + 16, :],
                    in0=tl[:], scalar1=MAGIC, scalar2=None,
                    op0=mybir.AluOpType.subtract)

        # panel-0 pre-block: everything Vector must finish for panel 0 and
        # panel 1, in readiness order, ahead of any PSUM-gated epilogue
        quant_x(0)
        quant_chunk(0, 0, tls0[0], wq_cur, warm=True)
        quant_x(1)
        for g in range(1, 8):
            quant_chunk(0, g, tls0[g], wq_cur, warm=True)
        for nb in range(2, NB):
            quant_x(nb)
        tls1 = load_panel(1)
        wq1 = [wtp.tile([P, 8, MP], BF16, tag="wq", name=f"wq1_{h}")
               for h in range(4)]
        for g in range(8):
            quant_chunk(1, g, tls1[g], wq1)

        # ---- main loop: 8 panels x 8 n-blocks x 32 k-steps ----
        wq_next, tls_next = wq1, tls1
        for p in range(NMP):
            if p >= 1 and p + 1 < NMP:
                # pre-block: next panel's quantize, ahead of epilogues
                tls_next = load_panel(p + 1)
                wq_next = [wtp.tile([P, 8, MP], BF16, tag="wq",
                                    name=f"wq{p + 1}_{h}") for h in range(4)]
                for g in range(8):
                    quant_chunk(p + 1, g, tls_next[g], wq_next)
            for nb in range(NB):
                ps = psp.tile([P, MP], F32, tag="ps", name=f"ps{p}_{nb}")
                for i in range(KB):
                    ks = (4 * nb + i) % KB
                    nc.tensor.matmul(
                        ps[:], xT[:, nb, ks, :],
                        wq_cur[ks // 8][:, ks % 8, :],
                        start=(i == 0), stop=(i == KB - 1),
                    )
                ob = obp.tile([P, MP], F32, tag="ob", name=f"ob{p}_{nb}")
                nc.vector.scalar_tensor_tensor(
                    out=ob[:], in0=ps[:], scalar=scl[:, 2:3],
                    in1=bias_bc[:, p * MP : (p + 1) * MP],
                    op0=mybir.AluOpType.mult, op1=mybir.AluOpType.add,
                )
                nc.scalar.dma_start(
                    out[nb * P : (nb + 1) * P, p * MP : (p + 1) * MP], ob[:]
                )
            wq_cur = wq_next


def _build():
    global _cached_nc
    if _cached_nc is not None:
        return _cached_nc
    nc = bacc.Bacc("TRN2", target_bir_lowering=False, debug=False,
                   num_devices=N_CORES)
    xs = nc.dram_tensor("xs", [K, NS], F32, kind="ExternalInput")
    wa = nc.dram_tensor("wa", [K, WS], F32, kind="ExternalInput")
    wf = nc.dram_tensor("wf", [K, M], F32, kind="ExternalInput")
    bias = nc.dram_tensor("bias", [M], F32, kind="ExternalInput")
    out = nc.dram_tensor("out", [NS, M], F32, kind="ExternalOutput")
    with tile.TileContext(nc) as tc:
        _body(nc, tc, xs.ap(), wa.ap(), wf.ap(), bias.ap(), out.ap())
    nc.compile()
    _cached_nc = nc
    return nc


def kernel(x, weight, bias, _trace=False, _trace_kwargs=None):
    x = np.asarray(x, dtype=np.float32)
    weight = np.asarray(weight, dtype=np.float32)
    bias = np.ascontiguousarray(np.asarray(bias, dtype=np.float32))
    assert x.shape == (N, K) and weight.shape == (M, K) and bias.shape == (M,)

    nc = _build()
    xt = x.T                              # [K, N] view
    wt = np.ascontiguousarray(weight.T)   # [K, M]
    in_maps = [
        {
            "xs": np.ascontiguousarray(xt[:, c * NS : (c + 1) * NS]),
            "wa": np.ascontiguousarray(wt[:, c * WS : (c + 1) * WS]),
            "wf": wt,
            "bias": bias,
        }
        for c in range(N_CORES)
    ]
    res = run_bass_kernel_spmd(
        nc, in_maps, core_ids=list(range(N_CORES)),
        trace=_trace, **(_trace_kwargs or {}),
    )
    out = np.concatenate([res.results[c]["out"] for c in range(N_CORES)], axis=0)
    if _trace:
        return out, res
    return out
